# revision 17
# baseline (speedup 1.0000x reference)
"""Multi-head attention (S=2048, B=2, D=1024, H=16) on 8 Trainium2 cores.

Sharding: tensor-parallel over heads (4 groups of 4 heads) x data-parallel
over batch (2). Core r handles batch r//4, heads [4*(r%4), 4*(r%4)+4).
Each core projects its 256 channels, runs attention for its 4 heads, applies
its slice of the output projection, and a ReduceScatter over each 4-core
batch group sums the partial outputs and leaves each core with a 512-row
slice of the final [2048, 1024] output.

All matmul operands are bf16 (fp32r measures ~2 cycles/column on this HW;
bf16 measures ~1), with fp32 PSUM accumulation. Softmax denominators come
free from an extra ones-column appended to V in the PV matmul. V's bias and
the output bias are folded out algebraically and added on the host.

The query blocks taper (512,512,512,256,128,128) so the trailing
ReduceScatter chunks shrink: the only collective that cannot overlap
compute is the final 128-row one.
"""
import sys

sys.path.insert(0, "/opt/trn_rl_repo")

import numpy as np
import ml_dtypes
import concourse.bacc as bacc
import concourse.mybir as mybir
from concourse import tile
from concourse.bass_utils import run_bass_kernel_spmd

dt = mybir.dt
AF = mybir.ActivationFunctionType
BF16 = ml_dtypes.bfloat16

S, B, D = 2048, 2, 1024
H, DK = 16, 64
NCORES = 8
HC = 4                 # heads per core
CH = HC * DK           # 256 local channels per core
SCALE = np.float32(1.0 / np.sqrt(DK))
GROUPS = [[0, 1, 2, 3], [4, 5, 6, 7]]

NKD = D // 128         # 8 contraction tiles for projections
NTK = S // 128         # 16 key tiles

# Tapered query blocks; each block is one ReduceScatter chunk.
TQS = [512, 512, 512, 512]
NB = len(TQS)
TQ0 = [sum(TQS[:i]) for i in range(NB)]
NSUB = [t // 128 for t in TQS]              # 128-row out subtiles per block
SUB0 = [t // 128 for t in TQ0]              # first subtile index of block


def build_nc():
    f32, bf16 = dt.float32, dt.bfloat16
    nc = bacc.Bacc("TRN2", target_bir_lowering=False, debug=False,
                   num_devices=NCORES)

    xq = nc.dram_tensor("xq_t", [D, S], bf16, kind="ExternalInput").ap()
    xk = nc.dram_tensor("xk_t", [D, S], bf16, kind="ExternalInput").ap()
    xv = nc.dram_tensor("xv_t", [D, S], bf16, kind="ExternalInput").ap()
    wq = nc.dram_tensor("wq_t", [D, CH], bf16, kind="ExternalInput").ap()
    wk = nc.dram_tensor("wk_t", [D, CH], bf16, kind="ExternalInput").ap()
    wv = nc.dram_tensor("wv_t", [D, CH], bf16, kind="ExternalInput").ap()
    wo = nc.dram_tensor("wo_t", [CH, D], bf16, kind="ExternalInput").ap()
    bq = nc.dram_tensor("bq", [2, 128], f32, kind="ExternalInput").ap()
    bk = nc.dram_tensor("bk", [2, 128], f32, kind="ExternalInput").ap()
    ones = nc.dram_tensor("ones", [128, HC], bf16, kind="ExternalInput").ap()
    # Chunk c covers global token rows [TQ0[c], TQ0[c]+TQS[c]);
    # group-rank j receives rows TQ0[c] + j*TQS[c]//4 onward.
    out_ext = nc.dram_tensor("out_rs", [S // 4, D], bf16,
                             kind="ExternalOutput").ap()

    with tile.TileContext(nc) as tc:
        with tc.tile_pool(name="const", bufs=1) as cp, \
             tc.tile_pool(name="stream", bufs=1) as sp, \
             tc.tile_pool(name="psum", bufs=1, space="PSUM") as pp, \
             tc.tile_pool(name="dram", bufs=1, space="DRAM") as dp:

            # ---- resident weights / biases (DMA in need-order: wq first) ----
            wq_sb = [cp.tile([128, CH], bf16, tag=f"wq{k}", name=f"wq{k}")
                     for k in range(NKD)]
            wk_sb = [cp.tile([128, CH], bf16, tag=f"wk{k}", name=f"wk{k}")
                     for k in range(NKD)]
            wv_sb = [cp.tile([128, CH], bf16, tag=f"wv{k}", name=f"wv{k}")
                     for k in range(NKD)]
            wo_sb = [cp.tile([128, D], bf16, tag=f"wo{k}", name=f"wo{k}")
                     for k in range(2)]
            bq_sb = [cp.tile([128, 1], f32, tag=f"bq{j}", name=f"bq{j}")
                     for j in range(2)]
            bk_sb = [cp.tile([128, 1], f32, tag=f"bk{j}", name=f"bk{j}")
                     for j in range(2)]
            for k in range(NKD):
                nc.scalar.dma_start(wq_sb[k][:], wq[k * 128:(k + 1) * 128, :])
            for j in range(2):
                nc.scalar.dma_start(bq_sb[j][:], bq[j].unsqueeze(1))
                nc.scalar.dma_start(bk_sb[j][:], bk[j].unsqueeze(1))
            for k in range(NKD):
                nc.scalar.dma_start(wk_sb[k][:], wk[k * 128:(k + 1) * 128, :])
            for k in range(NKD):
                nc.scalar.dma_start(wv_sb[k][:], wv[k * 128:(k + 1) * 128, :])
            for k in range(2):
                nc.scalar.dma_start(wo_sb[k][:], wo[k * 128:(k + 1) * 128, :])

            # ---- persistent activations ----
            qc = [cp.tile([128, S], bf16, tag=f"qc{j}", name=f"qc{j}")
                  for j in range(2)]
            kc = [cp.tile([128, S], bf16, tag=f"kc{j}", name=f"kc{j}")
                  for j in range(2)]
            # V tiles: [token128, 4*(64 V + 1 ones)] per key tile
            vt = [cp.tile([128, HC * (DK + 1)], bf16, tag=f"vt{t}",
                          name=f"vt{t}") for t in range(NTK)]
            ctx = [cp.tile([128, S], bf16, tag=f"ctx{j}", name=f"ctx{j}")
                   for j in range(2)]
            ones_sb = cp.tile([128, HC], bf16, tag="ones", name="ones_sb")
            nc.scalar.dma_start(ones_sb[:], ones[:])
            for t in range(NTK):
                vt_view = vt[t][:].rearrange("p (h c) -> p h c", h=HC)
                nc.vector.tensor_copy(vt_view[:, :, DK:DK + 1],
                                      ones_sb[:].unsqueeze(2))

            # ---- Q/K projections: psum[j-tile] [128ch, 512t] = sum_k
            #      wq[k][:, j]   (stationary) . xq[k, t512] (moving) ----
            TP = 512
            for x_dram, w_sb, b_sb, dst, dma_eng in (
                    (xq, wq_sb, bq_sb, qc, nc.sync),
                    (xk, wk_sb, bk_sb, kc, nc.scalar)):
                for th in range(2):            # halves of the token range
                    xts = []
                    for k in range(NKD):
                        for t in range(2):
                            xt = sp.tile([128, TP], bf16, tag="xs", bufs=20,
                                         name=f"xs{k}_{t}")
                            tq0 = (th * 2 + t) * TP
                            dma_eng.dma_start(
                                xt[:], x_dram[k * 128:(k + 1) * 128,
                                              tq0:tq0 + TP])
                            xts.append(xt)
                    for j in range(2):
                        ps = [pp.tile([128, TP], f32, tag="cx", bufs=2,
                                      name=f"pp{j}_{t}") for t in range(2)]
                        for k in range(NKD):
                            for t in range(2):
                                nc.tensor.matmul(
                                    ps[t][:],
                                    w_sb[k][:, j * 128:(j + 1) * 128],
                                    xts[2 * k + t][:],
                                    start=(k == 0), stop=(k == NKD - 1))
                        for t in range(2):
                            tq0 = (th * 2 + t) * TP
                            nc.scalar.activation(
                                dst[j][:, tq0:tq0 + TP], ps[t][:],
                                AF.Identity, bias=b_sb[j][:, 0:1])

            # ---- V projection: psum [128t, 256ch] = sum_k
            #      xv[k, t128] (stationary) . wv[k] (moving) ----
            for tt in range(4):                # big spans of 4 t-tiles
                xvts = []
                for k in range(NKD):
                    xvt_ = sp.tile([128, TP], bf16, tag="xs", bufs=20,
                                   name=f"xvt{k}")
                    nc.sync.dma_start(
                        xvt_[:], xv[k * 128:(k + 1) * 128,
                                    tt * TP:(tt + 1) * TP])
                    xvts.append(xvt_)
                for ts in range(4):
                    t = tt * 4 + ts
                    pv = pp.tile([128, CH], f32, tag="cx", bufs=2,
                                 name=f"pv{t}")
                    for k in range(NKD):
                        nc.tensor.matmul(
                            pv[:], xvts[k][:, ts * 128:(ts + 1) * 128],
                            wv_sb[k][:],
                            start=(k == 0), stop=(k == NKD - 1))
                    # strided copy into [128, 4, 65][:, :, 0:64]
                    dst_view = vt[t][:].rearrange("p (h c) -> p h c", h=HC)
                    src_view = pv[:].rearrange("p (h c) -> p h c", h=HC)
                    nc.vector.tensor_copy(dst_view[:, :, 0:DK], src_view)

            # ---- attention + output projection ----
            cc_ins = [dp.tile([TQS[c], D], bf16, tag=f"ccin{c}",
                              name=f"cc_in{c}") for c in range(NB)]
            cc_outs = [dp.tile([TQS[c] // 4, D], bf16, tag=f"ccout{c}",
                               name=f"cc_out{c}") for c in range(NB)]

            def emit_outproj_subtile(sub, chunk):
                """Out-projection + store for one 128-row output subtile."""
                t0 = sub * 128
                po = pp.tile([128, 1024], f32, tag="s1", bufs=3,
                             name=f"po{sub}")
                for e in range(2):
                    for dv in range(2):
                        nc.tensor.matmul(
                            po[:, e * 512:(e + 1) * 512],
                            ctx[dv][:, t0:t0 + 128],
                            wo_sb[dv][:, e * 512:(e + 1) * 512],
                            start=(dv == 0), stop=(dv == 1))
                osb = sp.tile([128, D], bf16, tag="ot", bufs=8,
                              name=f"ot{sub}")
                nc.vector.tensor_copy(osb[:], po[:])
                r0 = sub * 128 - TQ0[chunk]
                nc.sync.dma_start(cc_ins[chunk][r0:r0 + 128, :], osb[:])
                if sub + 1 == SUB0[chunk] + NSUB[chunk]:
                    # chunk complete: ReduceScatter it (overlaps the
                    # attention compute of the following blocks)
                    nc.gpsimd.collective_compute(
                        "ReduceScatter", mybir.AluOpType.add,
                        replica_groups=GROUPS,
                        ins=[cc_ins[chunk][:]], outs=[cc_outs[chunk][:]])

            for bi in range(NB):
                tq0, tqn = TQ0[bi], TQS[bi]
                for p in range(2):             # head pairs (2p, 2p+1)
                    cxf = [pp.tile([65, 512], f32, tag="cx", bufs=2,
                                   name=f"cx{p}_{h}") for h in range(2)]
                    cx = [c_[:, 0:tqn] for c_ in cxf]
                    for tk in range(NTK):
                        # previous block's out-projection, interleaved a few
                        # steps into this block so its ctx (behind the
                        # normalize chain) is ready when the PE reaches it
                        if bi > 0 and p == 0 and tk >= 4 and \
                                (tk - 4) % 3 == 0:
                            j = (tk - 4) // 3
                            if j < NSUB[bi - 1]:
                                emit_outproj_subtile(SUB0[bi - 1] + j, bi - 1)
                        # head h at col offset 512*h: every matmul PSUM
                        # output starts on a 2KB bank boundary
                        s1f = pp.tile([128, 1024], f32, tag="s1", bufs=3,
                                      name=f"s1{tk}")
                        etf = sp.tile([128, 1024], bf16, tag="et", bufs=6,
                                      name=f"et{tk}")
                        for h in range(2):      # adjacent -> row-pack overlap
                            r0 = h * 64
                            nc.tensor.matmul(
                                s1f[:, h * 512:h * 512 + tqn],
                                kc[p][r0:r0 + 64, tk * 128:(tk + 1) * 128],
                                qc[p][r0:r0 + 64, tq0:tq0 + tqn],
                                start=True, stop=True)
                        if tqn == 512:
                            nc.scalar.activation(etf[:], s1f[:], AF.Exp)
                        else:
                            for h in range(2):
                                nc.scalar.activation(
                                    etf[:, h * 512:h * 512 + tqn],
                                    s1f[:, h * 512:h * 512 + tqn], AF.Exp)
                        for h in range(2):
                            hl = p * 2 + h
                            nc.tensor.matmul(
                                cx[h][:],
                                vt[tk][:, hl * 65:(hl + 1) * 65],
                                etf[:, h * 512:h * 512 + tqn],
                                start=(tk == 0), stop=(tk == NTK - 1))
                    cxs = []
                    for h in range(2):
                        # evacuate both psums first so the cx slots free up
                        c_ = sp.tile([65, 512], f32, tag="cxs", bufs=4,
                                     name=f"cxs{p}_{h}")
                        nc.vector.tensor_copy(c_[:, 0:tqn], cx[h][:])
                        cxs.append(c_[:, 0:tqn])
                    for h in range(2):
                        den = sp.tile([1, 512], f32, tag="den", bufs=4,
                                      name=f"den{p}_{h}")
                        nc.vector.tensor_copy(den[:, 0:tqn], cxs[h][64:65, :])
                        rc = sp.tile([1, 512], f32, tag="rc", bufs=4,
                                     name=f"rc{p}_{h}")
                        nc.vector.reciprocal_approx_fast(rc[:, 0:tqn],
                                                         den[:, 0:tqn])
                        bc = sp.tile([64, 512], f32, tag="bc", bufs=4,
                                     name=f"bc{p}_{h}")
                        nc.gpsimd.partition_broadcast(bc[:, 0:tqn],
                                                      rc[:, 0:tqn])
                        nc.vector.tensor_mul(
                            ctx[p][h * 64:(h + 1) * 64, tq0:tq0 + tqn],
                            cxs[h][0:64, :], bc[:, 0:tqn])
            # last block's out-projection
            for j in range(NSUB[NB - 1]):
                emit_outproj_subtile(SUB0[NB - 1] + j, NB - 1)

            # final stores, force-scheduled at the very end so a store
            # waiting on its ReduceScatter never head-of-line-blocks the
            # sync DMA queue mid-kernel
            with tc.tile_wait_until(10):
                for c in range(NB):
                    o0 = TQ0[c] // 4
                    nc.sync.dma_start(out_ext[o0:o0 + TQS[c] // 4, :],
                                      cc_outs[c][:])

    nc.finalize()
    return nc


_NC = None


def _get_nc():
    global _NC
    if _NC is None:
        _NC = build_nc()
    return _NC


def make_in_maps(q, k, v, Wq, bq, Wk, bk, Wv, bv, Wo, bo):
    """Shard + precondition full inputs into per-core input maps."""
    xq_b = [np.ascontiguousarray(q[:, b, :].T).astype(BF16) for b in range(B)]
    xk_b = [np.ascontiguousarray(k[:, b, :].T).astype(BF16) for b in range(B)]
    xv_b = [np.ascontiguousarray(v[:, b, :].T).astype(BF16) for b in range(B)]
    in_maps = []
    for r in range(NCORES):
        b = r // 4
        g = r % 4
        ch = slice(g * CH, (g + 1) * CH)
        in_maps.append({
            "xq_t": xq_b[b], "xk_t": xk_b[b], "xv_t": xv_b[b],
            "wq_t": np.ascontiguousarray((Wq[ch, :] * SCALE).T).astype(BF16),
            "wk_t": np.ascontiguousarray(Wk[ch, :].T).astype(BF16),
            "wv_t": np.ascontiguousarray(Wv[ch, :].T).astype(BF16),
            "wo_t": np.ascontiguousarray(Wo[:, ch].T).astype(BF16),
            "bq": (bq[ch] * SCALE).reshape(2, 128).astype(np.float32),
            "bk": bk[ch].reshape(2, 128).astype(np.float32),
            "ones": np.ones((128, HC), dtype=BF16),
        })
    return in_maps


def assemble(results, Wo, bv, bo):
    """Gather per-core ReduceScatter slices into the full [S, B, D] output."""
    out = np.empty((S, B, D), dtype=np.float32)
    for r in range(NCORES):
        b = r // 4
        j = r % 4
        for c in range(NB):
            rows = TQS[c] // 4
            g0 = TQ0[c] + j * rows               # global token rows
            o0 = TQ0[c] // 4                     # rows within out_rs
            out[g0:g0 + rows, b, :] = \
                results[r]["out_rs"][o0:o0 + rows].astype(np.float32)
    out += (bo + Wo @ bv).astype(np.float32)
    return out


def run_sharded(inputs, trace=False):
    nc = _get_nc()
    in_maps = make_in_maps(**inputs)
    res = run_bass_kernel_spmd(nc, in_maps, list(range(NCORES)), trace=trace)
    full = assemble(res.results, np.asarray(inputs["Wo"], dtype=np.float32),
                    np.asarray(inputs["bv"], dtype=np.float32),
                    np.asarray(inputs["bo"], dtype=np.float32))
    return full, res


def kernel(**inputs) -> np.ndarray:
    inputs = {k_: np.asarray(v_, dtype=np.float32)
              for k_, v_ in inputs.items()}
    full, _ = run_sharded(inputs)
    return full


# revision 18
# speedup vs baseline: 1.0142x; 1.0142x over previous
"""Multi-head attention (S=2048, B=2, D=1024, H=16) on 8 Trainium2 cores.

Sharding: tensor-parallel over heads (4 groups of 4 heads) x data-parallel
over batch (2). Core r handles batch r//4, heads [4*(r%4), 4*(r%4)+4).
Each core projects its 256 channels, runs attention for its 4 heads, applies
its slice of the output projection, and a ReduceScatter over each 4-core
batch group sums the partial outputs and leaves each core with a 512-row
slice of the final [2048, 1024] output.

All matmul operands are bf16 (fp32r measures ~2 cycles/column on this HW;
bf16 measures ~1), with fp32 PSUM accumulation. Softmax denominators come
free from an extra ones-column appended to V in the PV matmul. V's bias and
the output bias are folded out algebraically and added on the host.

The query blocks taper (512,512,512,256,128,128) so the trailing
ReduceScatter chunks shrink: the only collective that cannot overlap
compute is the final 128-row one.
"""
import sys

sys.path.insert(0, "/opt/trn_rl_repo")

import numpy as np
import ml_dtypes
import concourse.bacc as bacc
import concourse.mybir as mybir
from concourse import tile
from concourse.bass_utils import run_bass_kernel_spmd

dt = mybir.dt
AF = mybir.ActivationFunctionType
BF16 = ml_dtypes.bfloat16

S, B, D = 2048, 2, 1024
H, DK = 16, 64
NCORES = 8
HC = 4                 # heads per core
CH = HC * DK           # 256 local channels per core
SCALE = np.float32(1.0 / np.sqrt(DK))
GROUPS = [[0, 1, 2, 3], [4, 5, 6, 7]]

NKD = D // 128         # 8 contraction tiles for projections
NTK = S // 128         # 16 key tiles

# Tapered query blocks; each block is one ReduceScatter chunk.
TQS = [512, 512, 512, 512]
NB = len(TQS)
TQ0 = [sum(TQS[:i]) for i in range(NB)]
NSUB = [t // 128 for t in TQS]              # 128-row out subtiles per block
SUB0 = [t // 128 for t in TQ0]              # first subtile index of block


def build_nc():
    f32, bf16 = dt.float32, dt.bfloat16
    nc = bacc.Bacc("TRN2", target_bir_lowering=False, debug=False,
                   num_devices=NCORES)

    xq = nc.dram_tensor("xq_t", [D, S], bf16, kind="ExternalInput").ap()
    xk = nc.dram_tensor("xk_t", [D, S], bf16, kind="ExternalInput").ap()
    xv = nc.dram_tensor("xv_t", [D, S], bf16, kind="ExternalInput").ap()
    wq = nc.dram_tensor("wq_t", [D, CH], bf16, kind="ExternalInput").ap()
    wk = nc.dram_tensor("wk_t", [D, CH], bf16, kind="ExternalInput").ap()
    wv = nc.dram_tensor("wv_t", [D, CH], bf16, kind="ExternalInput").ap()
    wo = nc.dram_tensor("wo_t", [CH, D], bf16, kind="ExternalInput").ap()
    bq = nc.dram_tensor("bq", [2, 128], f32, kind="ExternalInput").ap()
    bk = nc.dram_tensor("bk", [2, 128], f32, kind="ExternalInput").ap()
    ones = nc.dram_tensor("ones", [128, HC], bf16, kind="ExternalInput").ap()
    # Chunk c covers global token rows [TQ0[c], TQ0[c]+TQS[c]);
    # group-rank j receives rows TQ0[c] + j*TQS[c]//4 onward.
    out_ext = nc.dram_tensor("out_rs", [S // 4, D], bf16,
                             kind="ExternalOutput").ap()

    with tile.TileContext(nc) as tc:
        with tc.tile_pool(name="const", bufs=1) as cp, \
             tc.tile_pool(name="stream", bufs=1) as sp, \
             tc.tile_pool(name="psum", bufs=1, space="PSUM") as pp, \
             tc.tile_pool(name="dram", bufs=1, space="DRAM") as dp:

            # ---- resident weights / biases (DMA in need-order: wq first) ----
            wq_sb = [cp.tile([128, CH], bf16, tag=f"wq{k}", name=f"wq{k}")
                     for k in range(NKD)]
            wk_sb = [cp.tile([128, CH], bf16, tag=f"wk{k}", name=f"wk{k}")
                     for k in range(NKD)]
            wv_sb = [cp.tile([128, CH], bf16, tag=f"wv{k}", name=f"wv{k}")
                     for k in range(NKD)]
            wo_sb = [cp.tile([128, D], bf16, tag=f"wo{k}", name=f"wo{k}")
                     for k in range(2)]
            bq_sb = [cp.tile([128, 1], f32, tag=f"bq{j}", name=f"bq{j}")
                     for j in range(2)]
            bk_sb = [cp.tile([128, 1], f32, tag=f"bk{j}", name=f"bk{j}")
                     for j in range(2)]
            for k in range(NKD):
                nc.scalar.dma_start(wq_sb[k][:], wq[k * 128:(k + 1) * 128, :])
            for j in range(2):
                nc.scalar.dma_start(bq_sb[j][:], bq[j].unsqueeze(1))
                nc.scalar.dma_start(bk_sb[j][:], bk[j].unsqueeze(1))
            for k in range(NKD):
                nc.scalar.dma_start(wk_sb[k][:], wk[k * 128:(k + 1) * 128, :])
            for k in range(NKD):
                nc.scalar.dma_start(wv_sb[k][:], wv[k * 128:(k + 1) * 128, :])
            for k in range(2):
                nc.scalar.dma_start(wo_sb[k][:], wo[k * 128:(k + 1) * 128, :])

            # ---- persistent activations ----
            qc = [cp.tile([128, S], bf16, tag=f"qc{j}", name=f"qc{j}")
                  for j in range(2)]
            kc = [cp.tile([128, S], bf16, tag=f"kc{j}", name=f"kc{j}")
                  for j in range(2)]
            # V tiles: [token128, 4*(64 V + 1 ones)] per key tile
            vt = [cp.tile([128, HC * (DK + 1)], bf16, tag=f"vt{t}",
                          name=f"vt{t}") for t in range(NTK)]
            ctx = [cp.tile([128, S], bf16, tag=f"ctx{j}", name=f"ctx{j}")
                   for j in range(2)]
            ones_sb = cp.tile([128, HC], bf16, tag="ones", name="ones_sb")
            nc.scalar.dma_start(ones_sb[:], ones[:])
            for t in range(NTK):
                vt_view = vt[t][:].rearrange("p (h c) -> p h c", h=HC)
                nc.vector.tensor_copy(vt_view[:, :, DK:DK + 1],
                                      ones_sb[:].unsqueeze(2))

            # ---- Q/K projections: psum[j-tile] [128ch, 512t] = sum_k
            #      wq[k][:, j]   (stationary) . xq[k, t512] (moving) ----
            TP = 512
            for x_dram, w_sb, b_sb, dst, dma_eng in (
                    (xq, wq_sb, bq_sb, qc, nc.sync),
                    (xk, wk_sb, bk_sb, kc, nc.scalar)):
                for th in range(2):            # halves of the token range
                    xts = []
                    for k in range(NKD):
                        for t in range(2):
                            xt = sp.tile([128, TP], bf16, tag="xs", bufs=20,
                                         name=f"xs{k}_{t}")
                            tq0 = (th * 2 + t) * TP
                            dma_eng.dma_start(
                                xt[:], x_dram[k * 128:(k + 1) * 128,
                                              tq0:tq0 + TP])
                            xts.append(xt)
                    for j in range(2):
                        ps = [pp.tile([128, TP], f32, tag="cx", bufs=2,
                                      name=f"pp{j}_{t}") for t in range(2)]
                        for k in range(NKD):
                            for t in range(2):
                                nc.tensor.matmul(
                                    ps[t][:],
                                    w_sb[k][:, j * 128:(j + 1) * 128],
                                    xts[2 * k + t][:],
                                    start=(k == 0), stop=(k == NKD - 1))
                        for t in range(2):
                            tq0 = (th * 2 + t) * TP
                            nc.scalar.activation(
                                dst[j][:, tq0:tq0 + TP], ps[t][:],
                                AF.Identity, bias=b_sb[j][:, 0:1])

            # ---- V projection: psum [128t, 256ch] = sum_k
            #      xv[k, t128] (stationary) . wv[k] (moving) ----
            for tt in range(4):                # big spans of 4 t-tiles
                xvts = []
                for k in range(NKD):
                    xvt_ = sp.tile([128, TP], bf16, tag="xs", bufs=20,
                                   name=f"xvt{k}")
                    nc.sync.dma_start(
                        xvt_[:], xv[k * 128:(k + 1) * 128,
                                    tt * TP:(tt + 1) * TP])
                    xvts.append(xvt_)
                for ts in range(4):
                    t = tt * 4 + ts
                    pv = pp.tile([128, CH], f32, tag="cx", bufs=2,
                                 name=f"pv{t}")
                    for k in range(NKD):
                        nc.tensor.matmul(
                            pv[:], xvts[k][:, ts * 128:(ts + 1) * 128],
                            wv_sb[k][:],
                            start=(k == 0), stop=(k == NKD - 1))
                    # strided copy into [128, 4, 65][:, :, 0:64]
                    dst_view = vt[t][:].rearrange("p (h c) -> p h c", h=HC)
                    src_view = pv[:].rearrange("p (h c) -> p h c", h=HC)
                    nc.vector.tensor_copy(dst_view[:, :, 0:DK], src_view)

            # ---- attention + output projection ----
            cc_ins = [dp.tile([TQS[c], D], bf16, tag=f"ccin{c}",
                              name=f"cc_in{c}") for c in range(NB)]
            cc_outs = [dp.tile([TQS[c] // 4, D], bf16, tag=f"ccout{c}",
                               name=f"cc_out{c}") for c in range(NB)]

            def emit_outproj_subtile(sub, chunk):
                """Out-projection + store for one 128-row output subtile."""
                t0 = sub * 128
                po = pp.tile([128, 1024], f32, tag="s1", bufs=3,
                             name=f"po{sub}")
                for e in range(2):
                    for dv in range(2):
                        nc.tensor.matmul(
                            po[:, e * 512:(e + 1) * 512],
                            ctx[dv][:, t0:t0 + 128],
                            wo_sb[dv][:, e * 512:(e + 1) * 512],
                            start=(dv == 0), stop=(dv == 1))
                osb = sp.tile([128, D], bf16, tag="ot", bufs=8,
                              name=f"ot{sub}")
                nc.vector.tensor_copy(osb[:], po[:])
                r0 = sub * 128 - TQ0[chunk]
                nc.sync.dma_start(cc_ins[chunk][r0:r0 + 128, :], osb[:])
                if sub + 1 == SUB0[chunk] + NSUB[chunk]:
                    # chunk complete: ReduceScatter it (overlaps the
                    # attention compute of the following blocks)
                    nc.gpsimd.collective_compute(
                        "ReduceScatter", mybir.AluOpType.add,
                        replica_groups=GROUPS,
                        ins=[cc_ins[chunk][:]], outs=[cc_outs[chunk][:]])

            # Flattened attention stream over all (block, head-pair)
            # steps, with each PV pair deferred PV_LAG steps behind its
            # scores/exp. The deferral crosses pair boundaries, so the
            # next pair's scores+exp issue before the previous pair's
            # last PV and its cx-PSUM evacuation latency stays off the
            # ACT engine's critical path.
            PV_LAG = 2

            def emit_normalize(bi, p, cx):
                tq0, tqn = TQ0[bi], TQS[bi]
                cxs = []
                for h in range(2):
                    c_ = sp.tile([65, 512], f32, tag="cxs", bufs=4,
                                 name=f"cxs{p}_{h}")
                    nc.vector.tensor_copy(c_[:, 0:tqn], cx[h][:])
                    cxs.append(c_[:, 0:tqn])
                for h in range(2):
                    den = sp.tile([1, 512], f32, tag="den", bufs=4,
                                  name=f"den{p}_{h}")
                    nc.vector.tensor_copy(den[:, 0:tqn], cxs[h][64:65, :])
                    rc = sp.tile([1, 512], f32, tag="rc", bufs=4,
                                 name=f"rc{p}_{h}")
                    nc.vector.reciprocal_approx_fast(rc[:, 0:tqn],
                                                     den[:, 0:tqn])
                    bc = sp.tile([64, 512], f32, tag="bc", bufs=4,
                                 name=f"bc{p}_{h}")
                    nc.gpsimd.partition_broadcast(bc[:, 0:tqn],
                                                  rc[:, 0:tqn])
                    nc.vector.tensor_mul(
                        ctx[p][h * 64:(h + 1) * 64, tq0:tq0 + tqn],
                        cxs[h][0:64, :], bc[:, 0:tqn])

            pvq = []        # deferred PV steps: (bi, p, tk, etf, cx)

            def pop_pv():
                bi_, p_, tk_, etf_, cx_ = pvq.pop(0)
                tqn_ = TQS[bi_]
                for h in range(2):
                    hl = p_ * 2 + h
                    nc.tensor.matmul(
                        cx_[h][:],
                        vt[tk_][:, hl * 65:(hl + 1) * 65],
                        etf_[:, h * 512:h * 512 + tqn_],
                        start=(tk_ == 0), stop=(tk_ == NTK - 1))
                if tk_ == NTK - 1:
                    emit_normalize(bi_, p_, cx_)

            for bi in range(NB):
                tq0, tqn = TQ0[bi], TQS[bi]
                for p in range(2):             # head pairs (2p, 2p+1)
                    cxf = [pp.tile([65, 512], f32, tag="cx", bufs=2,
                                   name=f"cx{p}_{h}") for h in range(2)]
                    cx = [c_[:, 0:tqn] for c_ in cxf]
                    for tk in range(NTK):
                        # previous block's out-projection, interleaved a few
                        # steps into this block so its ctx (behind the
                        # normalize chain) is ready when the PE reaches it
                        if bi > 0 and p == 0 and tk >= 4 and \
                                (tk - 4) % 3 == 0:
                            j = (tk - 4) // 3
                            if j < NSUB[bi - 1]:
                                emit_outproj_subtile(SUB0[bi - 1] + j, bi - 1)
                        # head h at col offset 512*h: every matmul PSUM
                        # output starts on a 2KB bank boundary
                        s1f = pp.tile([128, 1024], f32, tag="s1", bufs=3,
                                      name=f"s1{tk}")
                        etf = sp.tile([128, 1024], bf16, tag="et", bufs=6,
                                      name=f"et{tk}")
                        for h in range(2):      # adjacent -> row-pack overlap
                            r0 = h * 64
                            nc.tensor.matmul(
                                s1f[:, h * 512:h * 512 + tqn],
                                kc[p][r0:r0 + 64, tk * 128:(tk + 1) * 128],
                                qc[p][r0:r0 + 64, tq0:tq0 + tqn],
                                start=True, stop=True)
                        if tqn == 512:
                            nc.scalar.activation(etf[:], s1f[:], AF.Exp)
                        else:
                            for h in range(2):
                                nc.scalar.activation(
                                    etf[:, h * 512:h * 512 + tqn],
                                    s1f[:, h * 512:h * 512 + tqn], AF.Exp)
                        pvq.append((bi, p, tk, etf, cx))
                        while len(pvq) > PV_LAG:
                            pop_pv()
            while pvq:
                pop_pv()
            # last block's out-projection
            for j in range(NSUB[NB - 1]):
                emit_outproj_subtile(SUB0[NB - 1] + j, NB - 1)

            # final stores, force-scheduled at the very end so a store
            # waiting on its ReduceScatter never head-of-line-blocks the
            # sync DMA queue mid-kernel
            with tc.tile_wait_until(10):
                for c in range(NB):
                    o0 = TQ0[c] // 4
                    nc.sync.dma_start(out_ext[o0:o0 + TQS[c] // 4, :],
                                      cc_outs[c][:])

    nc.finalize()
    return nc


_NC = None


def _get_nc():
    global _NC
    if _NC is None:
        _NC = build_nc()
    return _NC


def make_in_maps(q, k, v, Wq, bq, Wk, bk, Wv, bv, Wo, bo):
    """Shard + precondition full inputs into per-core input maps."""
    xq_b = [np.ascontiguousarray(q[:, b, :].T).astype(BF16) for b in range(B)]
    xk_b = [np.ascontiguousarray(k[:, b, :].T).astype(BF16) for b in range(B)]
    xv_b = [np.ascontiguousarray(v[:, b, :].T).astype(BF16) for b in range(B)]
    in_maps = []
    for r in range(NCORES):
        b = r // 4
        g = r % 4
        ch = slice(g * CH, (g + 1) * CH)
        in_maps.append({
            "xq_t": xq_b[b], "xk_t": xk_b[b], "xv_t": xv_b[b],
            "wq_t": np.ascontiguousarray((Wq[ch, :] * SCALE).T).astype(BF16),
            "wk_t": np.ascontiguousarray(Wk[ch, :].T).astype(BF16),
            "wv_t": np.ascontiguousarray(Wv[ch, :].T).astype(BF16),
            "wo_t": np.ascontiguousarray(Wo[:, ch].T).astype(BF16),
            "bq": (bq[ch] * SCALE).reshape(2, 128).astype(np.float32),
            "bk": bk[ch].reshape(2, 128).astype(np.float32),
            "ones": np.ones((128, HC), dtype=BF16),
        })
    return in_maps


def assemble(results, Wo, bv, bo):
    """Gather per-core ReduceScatter slices into the full [S, B, D] output."""
    out = np.empty((S, B, D), dtype=np.float32)
    for r in range(NCORES):
        b = r // 4
        j = r % 4
        for c in range(NB):
            rows = TQS[c] // 4
            g0 = TQ0[c] + j * rows               # global token rows
            o0 = TQ0[c] // 4                     # rows within out_rs
            out[g0:g0 + rows, b, :] = \
                results[r]["out_rs"][o0:o0 + rows].astype(np.float32)
    out += (bo + Wo @ bv).astype(np.float32)
    return out


def run_sharded(inputs, trace=False):
    nc = _get_nc()
    in_maps = make_in_maps(**inputs)
    res = run_bass_kernel_spmd(nc, in_maps, list(range(NCORES)), trace=trace)
    full = assemble(res.results, np.asarray(inputs["Wo"], dtype=np.float32),
                    np.asarray(inputs["bv"], dtype=np.float32),
                    np.asarray(inputs["bo"], dtype=np.float32))
    return full, res


def kernel(**inputs) -> np.ndarray:
    inputs = {k_: np.asarray(v_, dtype=np.float32)
              for k_, v_ in inputs.items()}
    full, _ = run_sharded(inputs)
    return full


# revision 19
# speedup vs baseline: 1.0195x; 1.0052x over previous
"""Multi-head attention (S=2048, B=2, D=1024, H=16) on 8 Trainium2 cores.

Sharding: tensor-parallel over heads (4 groups of 4 heads) x data-parallel
over batch (2). Core r handles batch r//4, heads [4*(r%4), 4*(r%4)+4).
Each core projects its 256 channels, runs attention for its 4 heads, applies
its slice of the output projection, and a ReduceScatter over each 4-core
batch group sums the partial outputs and leaves each core with a 512-row
slice of the final [2048, 1024] output.

All matmul operands are bf16 (fp32r measures ~2 cycles/column on this HW;
bf16 measures ~1), with fp32 PSUM accumulation. Softmax denominators come
free from an extra ones-column appended to V in the PV matmul. V's bias and
the output bias are folded out algebraically and added on the host.

The query blocks taper (512,512,512,256,128,128) so the trailing
ReduceScatter chunks shrink: the only collective that cannot overlap
compute is the final 128-row one.
"""
import sys

sys.path.insert(0, "/opt/trn_rl_repo")

import numpy as np
import ml_dtypes
import concourse.bacc as bacc
import concourse.mybir as mybir
from concourse import tile
from concourse.bass_utils import run_bass_kernel_spmd

dt = mybir.dt
AF = mybir.ActivationFunctionType
BF16 = ml_dtypes.bfloat16

S, B, D = 2048, 2, 1024
H, DK = 16, 64
NCORES = 8
HC = 4                 # heads per core
CH = HC * DK           # 256 local channels per core
SCALE = np.float32(1.0 / np.sqrt(DK))
GROUPS = [[0, 1, 2, 3], [4, 5, 6, 7]]

NKD = D // 128         # 8 contraction tiles for projections
NTK = S // 128         # 16 key tiles

# Tapered query blocks; each block is one ReduceScatter chunk.
TQS = [512, 512, 512, 512]
NB = len(TQS)
TQ0 = [sum(TQS[:i]) for i in range(NB)]
NSUB = [t // 128 for t in TQS]              # 128-row out subtiles per block
SUB0 = [t // 128 for t in TQ0]              # first subtile index of block


def build_nc():
    f32, bf16 = dt.float32, dt.bfloat16
    nc = bacc.Bacc("TRN2", target_bir_lowering=False, debug=False,
                   num_devices=NCORES)

    xq = nc.dram_tensor("xq_t", [D, S], bf16, kind="ExternalInput").ap()
    xk = nc.dram_tensor("xk_t", [D, S], bf16, kind="ExternalInput").ap()
    xv = nc.dram_tensor("xv_t", [D, S], bf16, kind="ExternalInput").ap()
    wq = nc.dram_tensor("wq_t", [D, CH], bf16, kind="ExternalInput").ap()
    wk = nc.dram_tensor("wk_t", [D, CH], bf16, kind="ExternalInput").ap()
    wv = nc.dram_tensor("wv_t", [D, CH], bf16, kind="ExternalInput").ap()
    wo = nc.dram_tensor("wo_t", [CH, D], bf16, kind="ExternalInput").ap()
    bq = nc.dram_tensor("bq", [2, 128], f32, kind="ExternalInput").ap()
    bk = nc.dram_tensor("bk", [2, 128], f32, kind="ExternalInput").ap()
    ones = nc.dram_tensor("ones", [128, HC], bf16, kind="ExternalInput").ap()
    # Chunk c covers global token rows [TQ0[c], TQ0[c]+TQS[c]);
    # group-rank j receives rows TQ0[c] + j*TQS[c]//4 onward.
    out_ext = nc.dram_tensor("out_rs", [S // 4, D], bf16,
                             kind="ExternalOutput").ap()

    with tile.TileContext(nc) as tc:
        with tc.tile_pool(name="const", bufs=1) as cp, \
             tc.tile_pool(name="stream", bufs=1) as sp, \
             tc.tile_pool(name="psum", bufs=1, space="PSUM") as pp, \
             tc.tile_pool(name="dram", bufs=1, space="DRAM") as dp:

            # ---- resident weights / biases (DMA in need-order: wq first) ----
            wq_sb = [cp.tile([128, CH], bf16, tag=f"wq{k}", name=f"wq{k}")
                     for k in range(NKD)]
            wk_sb = [cp.tile([128, CH], bf16, tag=f"wk{k}", name=f"wk{k}")
                     for k in range(NKD)]
            wv_sb = [cp.tile([128, CH], bf16, tag=f"wv{k}", name=f"wv{k}")
                     for k in range(NKD)]
            wo_sb = [cp.tile([128, D], bf16, tag=f"wo{k}", name=f"wo{k}")
                     for k in range(2)]
            bq_sb = [cp.tile([128, 1], f32, tag=f"bq{j}", name=f"bq{j}")
                     for j in range(2)]
            bk_sb = [cp.tile([128, 1], f32, tag=f"bk{j}", name=f"bk{j}")
                     for j in range(2)]
            for k in range(NKD):
                nc.scalar.dma_start(wq_sb[k][:], wq[k * 128:(k + 1) * 128, :])
            for j in range(2):
                nc.scalar.dma_start(bq_sb[j][:], bq[j].unsqueeze(1))
                nc.scalar.dma_start(bk_sb[j][:], bk[j].unsqueeze(1))
            for k in range(NKD):
                nc.scalar.dma_start(wk_sb[k][:], wk[k * 128:(k + 1) * 128, :])
            for k in range(NKD):
                nc.scalar.dma_start(wv_sb[k][:], wv[k * 128:(k + 1) * 128, :])
            for k in range(2):
                nc.scalar.dma_start(wo_sb[k][:], wo[k * 128:(k + 1) * 128, :])

            # ---- persistent activations ----
            qc = [cp.tile([128, S], bf16, tag=f"qc{j}", name=f"qc{j}")
                  for j in range(2)]
            kc = [cp.tile([128, S], bf16, tag=f"kc{j}", name=f"kc{j}")
                  for j in range(2)]
            # V tiles: [token128, 4*(64 V + 1 ones)] per key tile
            vt = [cp.tile([128, HC * (DK + 1)], bf16, tag=f"vt{t}",
                          name=f"vt{t}") for t in range(NTK)]
            ctx = [cp.tile([128, S], bf16, tag=f"ctx{j}", name=f"ctx{j}")
                   for j in range(2)]
            ones_sb = cp.tile([128, HC], bf16, tag="ones", name="ones_sb")
            nc.scalar.dma_start(ones_sb[:], ones[:])
            for t in range(NTK):
                vt_view = vt[t][:].rearrange("p (h c) -> p h c", h=HC)
                nc.vector.tensor_copy(vt_view[:, :, DK:DK + 1],
                                      ones_sb[:].unsqueeze(2))

            # ---- Q/K projections: psum[j-tile] [128ch, 512t] = sum_k
            #      wq[k][:, j]   (stationary) . xq[k, t512] (moving) ----
            TP = 512
            for x_dram, w_sb, b_sb, dst, dma_eng in (
                    (xq, wq_sb, bq_sb, qc, nc.sync),
                    (xk, wk_sb, bk_sb, kc, nc.scalar)):
                for th in range(2):            # halves of the token range
                    xts = []
                    for k in range(NKD):
                        for t in range(2):
                            xt = sp.tile([128, TP], bf16, tag="xs", bufs=20,
                                         name=f"xs{k}_{t}")
                            tq0 = (th * 2 + t) * TP
                            dma_eng.dma_start(
                                xt[:], x_dram[k * 128:(k + 1) * 128,
                                              tq0:tq0 + TP])
                            xts.append(xt)
                    for j in range(2):
                        ps = [pp.tile([128, TP], f32, tag="cx", bufs=2,
                                      name=f"pp{j}_{t}") for t in range(2)]
                        for k in range(NKD):
                            for t in range(2):
                                nc.tensor.matmul(
                                    ps[t][:],
                                    w_sb[k][:, j * 128:(j + 1) * 128],
                                    xts[2 * k + t][:],
                                    start=(k == 0), stop=(k == NKD - 1))
                        for t in range(2):
                            tq0 = (th * 2 + t) * TP
                            nc.scalar.activation(
                                dst[j][:, tq0:tq0 + TP], ps[t][:],
                                AF.Identity, bias=b_sb[j][:, 0:1])

            # ---- V projection: psum [128t, 256ch] = sum_k
            #      xv[k, t128] (stationary) . wv[k] (moving) ----
            for tt in range(4):                # big spans of 4 t-tiles
                xvts = []
                for k in range(NKD):
                    xvt_ = sp.tile([128, TP], bf16, tag="xs", bufs=20,
                                   name=f"xvt{k}")
                    nc.sync.dma_start(
                        xvt_[:], xv[k * 128:(k + 1) * 128,
                                    tt * TP:(tt + 1) * TP])
                    xvts.append(xvt_)
                for ts in range(4):
                    t = tt * 4 + ts
                    pv = pp.tile([128, CH], f32, tag="cx", bufs=2,
                                 name=f"pv{t}")
                    for k in range(NKD):
                        nc.tensor.matmul(
                            pv[:], xvts[k][:, ts * 128:(ts + 1) * 128],
                            wv_sb[k][:],
                            start=(k == 0), stop=(k == NKD - 1))
                    # strided copy into [128, 4, 65][:, :, 0:64]
                    dst_view = vt[t][:].rearrange("p (h c) -> p h c", h=HC)
                    src_view = pv[:].rearrange("p (h c) -> p h c", h=HC)
                    nc.vector.tensor_copy(dst_view[:, :, 0:DK], src_view)

            # ---- attention + output projection ----
            # Warmup collective: the first CC op on the stream pays a
            # ~15-25us warmup penalty; absorb it with a tiny dummy
            # ReduceScatter (contents irrelevant, output unused) issued at
            # kernel start so the real per-chunk ops run at steady state.
            cc_warm_in = dp.tile([16, D], bf16, tag="ccwi", name="cc_warm_in")
            cc_warm_out = dp.tile([4, D], bf16, tag="ccwo",
                                  name="cc_warm_out")
            nc.gpsimd.collective_compute(
                "ReduceScatter", mybir.AluOpType.add,
                replica_groups=GROUPS,
                ins=[cc_warm_in[:]], outs=[cc_warm_out[:]])
            cc_ins = [dp.tile([TQS[c], D], bf16, tag=f"ccin{c}",
                              name=f"cc_in{c}") for c in range(NB)]
            cc_outs = [dp.tile([TQS[c] // 4, D], bf16, tag=f"ccout{c}",
                               name=f"cc_out{c}") for c in range(NB)]

            def emit_outproj_subtile(sub, chunk):
                """Out-projection + store for one 128-row output subtile."""
                t0 = sub * 128
                po = pp.tile([128, 1024], f32, tag="s1", bufs=3,
                             name=f"po{sub}")
                for e in range(2):
                    for dv in range(2):
                        nc.tensor.matmul(
                            po[:, e * 512:(e + 1) * 512],
                            ctx[dv][:, t0:t0 + 128],
                            wo_sb[dv][:, e * 512:(e + 1) * 512],
                            start=(dv == 0), stop=(dv == 1))
                osb = sp.tile([128, D], bf16, tag="ot", bufs=8,
                              name=f"ot{sub}")
                nc.vector.tensor_copy(osb[:], po[:])
                r0 = sub * 128 - TQ0[chunk]
                nc.sync.dma_start(cc_ins[chunk][r0:r0 + 128, :], osb[:])
                if sub + 1 == SUB0[chunk] + NSUB[chunk]:
                    # chunk complete: ReduceScatter it (overlaps the
                    # attention compute of the following blocks)
                    nc.gpsimd.collective_compute(
                        "ReduceScatter", mybir.AluOpType.add,
                        replica_groups=GROUPS,
                        ins=[cc_ins[chunk][:]], outs=[cc_outs[chunk][:]])

            # Flattened attention stream over all (block, head-pair)
            # steps, with each PV pair deferred PV_LAG steps behind its
            # scores/exp. The deferral crosses pair boundaries, so the
            # next pair's scores+exp issue before the previous pair's
            # last PV and its cx-PSUM evacuation latency stays off the
            # ACT engine's critical path.
            PV_LAG = 2

            def emit_normalize(bi, p, cx):
                tq0, tqn = TQ0[bi], TQS[bi]
                cxs = []
                for h in range(2):
                    c_ = sp.tile([65, 512], f32, tag="cxs", bufs=4,
                                 name=f"cxs{p}_{h}")
                    nc.vector.tensor_copy(c_[:, 0:tqn], cx[h][:])
                    cxs.append(c_[:, 0:tqn])
                for h in range(2):
                    den = sp.tile([1, 512], f32, tag="den", bufs=4,
                                  name=f"den{p}_{h}")
                    nc.vector.tensor_copy(den[:, 0:tqn], cxs[h][64:65, :])
                    rc = sp.tile([1, 512], f32, tag="rc", bufs=4,
                                 name=f"rc{p}_{h}")
                    nc.vector.reciprocal_approx_fast(rc[:, 0:tqn],
                                                     den[:, 0:tqn])
                    bc = sp.tile([64, 512], f32, tag="bc", bufs=4,
                                 name=f"bc{p}_{h}")
                    nc.gpsimd.partition_broadcast(bc[:, 0:tqn],
                                                  rc[:, 0:tqn])
                    nc.vector.tensor_mul(
                        ctx[p][h * 64:(h + 1) * 64, tq0:tq0 + tqn],
                        cxs[h][0:64, :], bc[:, 0:tqn])

            pvq = []        # deferred PV steps: (bi, p, tk, etf, cx)

            def pop_pv():
                bi_, p_, tk_, etf_, cx_ = pvq.pop(0)
                tqn_ = TQS[bi_]
                for h in range(2):
                    hl = p_ * 2 + h
                    nc.tensor.matmul(
                        cx_[h][:],
                        vt[tk_][:, hl * 65:(hl + 1) * 65],
                        etf_[:, h * 512:h * 512 + tqn_],
                        start=(tk_ == 0), stop=(tk_ == NTK - 1))
                if tk_ == NTK - 1:
                    emit_normalize(bi_, p_, cx_)

            for bi in range(NB):
                tq0, tqn = TQ0[bi], TQS[bi]
                for p in range(2):             # head pairs (2p, 2p+1)
                    cxf = [pp.tile([65, 512], f32, tag="cx", bufs=2,
                                   name=f"cx{p}_{h}") for h in range(2)]
                    cx = [c_[:, 0:tqn] for c_ in cxf]
                    for tk in range(NTK):
                        # previous block's out-projection, interleaved a few
                        # steps into this block so its ctx (behind the
                        # normalize chain) is ready when the PE reaches it
                        if bi > 0 and p == 0 and tk >= 4 and \
                                (tk - 4) % 3 == 0:
                            j = (tk - 4) // 3
                            if j < NSUB[bi - 1]:
                                emit_outproj_subtile(SUB0[bi - 1] + j, bi - 1)
                        # head h at col offset 512*h: every matmul PSUM
                        # output starts on a 2KB bank boundary
                        s1f = pp.tile([128, 1024], f32, tag="s1", bufs=3,
                                      name=f"s1{tk}")
                        etf = sp.tile([128, 1024], bf16, tag="et", bufs=6,
                                      name=f"et{tk}")
                        for h in range(2):      # adjacent -> row-pack overlap
                            r0 = h * 64
                            nc.tensor.matmul(
                                s1f[:, h * 512:h * 512 + tqn],
                                kc[p][r0:r0 + 64, tk * 128:(tk + 1) * 128],
                                qc[p][r0:r0 + 64, tq0:tq0 + tqn],
                                start=True, stop=True)
                        if tqn == 512:
                            nc.scalar.activation(etf[:], s1f[:], AF.Exp)
                        else:
                            for h in range(2):
                                nc.scalar.activation(
                                    etf[:, h * 512:h * 512 + tqn],
                                    s1f[:, h * 512:h * 512 + tqn], AF.Exp)
                        pvq.append((bi, p, tk, etf, cx))
                        while len(pvq) > PV_LAG:
                            pop_pv()
            while pvq:
                pop_pv()
            # last block's out-projection
            for j in range(NSUB[NB - 1]):
                emit_outproj_subtile(SUB0[NB - 1] + j, NB - 1)

            # final stores, force-scheduled at the very end so a store
            # waiting on its ReduceScatter never head-of-line-blocks the
            # sync DMA queue mid-kernel
            with tc.tile_wait_until(10):
                for c in range(NB):
                    o0 = TQ0[c] // 4
                    nc.sync.dma_start(out_ext[o0:o0 + TQS[c] // 4, :],
                                      cc_outs[c][:])

    nc.finalize()
    return nc


_NC = None


def _get_nc():
    global _NC
    if _NC is None:
        _NC = build_nc()
    return _NC


def make_in_maps(q, k, v, Wq, bq, Wk, bk, Wv, bv, Wo, bo):
    """Shard + precondition full inputs into per-core input maps."""
    xq_b = [np.ascontiguousarray(q[:, b, :].T).astype(BF16) for b in range(B)]
    xk_b = [np.ascontiguousarray(k[:, b, :].T).astype(BF16) for b in range(B)]
    xv_b = [np.ascontiguousarray(v[:, b, :].T).astype(BF16) for b in range(B)]
    in_maps = []
    for r in range(NCORES):
        b = r // 4
        g = r % 4
        ch = slice(g * CH, (g + 1) * CH)
        in_maps.append({
            "xq_t": xq_b[b], "xk_t": xk_b[b], "xv_t": xv_b[b],
            "wq_t": np.ascontiguousarray((Wq[ch, :] * SCALE).T).astype(BF16),
            "wk_t": np.ascontiguousarray(Wk[ch, :].T).astype(BF16),
            "wv_t": np.ascontiguousarray(Wv[ch, :].T).astype(BF16),
            "wo_t": np.ascontiguousarray(Wo[:, ch].T).astype(BF16),
            "bq": (bq[ch] * SCALE).reshape(2, 128).astype(np.float32),
            "bk": bk[ch].reshape(2, 128).astype(np.float32),
            "ones": np.ones((128, HC), dtype=BF16),
        })
    return in_maps


def assemble(results, Wo, bv, bo):
    """Gather per-core ReduceScatter slices into the full [S, B, D] output."""
    out = np.empty((S, B, D), dtype=np.float32)
    for r in range(NCORES):
        b = r // 4
        j = r % 4
        for c in range(NB):
            rows = TQS[c] // 4
            g0 = TQ0[c] + j * rows               # global token rows
            o0 = TQ0[c] // 4                     # rows within out_rs
            out[g0:g0 + rows, b, :] = \
                results[r]["out_rs"][o0:o0 + rows].astype(np.float32)
    out += (bo + Wo @ bv).astype(np.float32)
    return out


def run_sharded(inputs, trace=False):
    nc = _get_nc()
    in_maps = make_in_maps(**inputs)
    res = run_bass_kernel_spmd(nc, in_maps, list(range(NCORES)), trace=trace)
    full = assemble(res.results, np.asarray(inputs["Wo"], dtype=np.float32),
                    np.asarray(inputs["bv"], dtype=np.float32),
                    np.asarray(inputs["bo"], dtype=np.float32))
    return full, res


def kernel(**inputs) -> np.ndarray:
    inputs = {k_: np.asarray(v_, dtype=np.float32)
              for k_, v_ in inputs.items()}
    full, _ = run_sharded(inputs)
    return full


# revision 20
# speedup vs baseline: 1.0264x; 1.0068x over previous
"""Multi-head attention (S=2048, B=2, D=1024, H=16) on 8 Trainium2 cores.

Sharding: tensor-parallel over heads (4 groups of 4 heads) x data-parallel
over batch (2). Core r handles batch r//4, heads [4*(r%4), 4*(r%4)+4).
Each core projects its 256 channels, runs attention for its 4 heads, applies
its slice of the output projection, and a ReduceScatter over each 4-core
batch group sums the partial outputs and leaves each core with a 512-row
slice of the final [2048, 1024] output.

All matmul operands are bf16 (fp32r measures ~2 cycles/column on this HW;
bf16 measures ~1), with fp32 PSUM accumulation. Softmax denominators come
free from an extra ones-column appended to V in the PV matmul. V's bias and
the output bias are folded out algebraically and added on the host.

The query blocks taper (512,512,512,256,128,128) so the trailing
ReduceScatter chunks shrink: the only collective that cannot overlap
compute is the final 128-row one.
"""
import sys

sys.path.insert(0, "/opt/trn_rl_repo")

import numpy as np
import ml_dtypes
import concourse.bacc as bacc
import concourse.mybir as mybir
from concourse import tile
from concourse.bass_utils import run_bass_kernel_spmd

dt = mybir.dt
AF = mybir.ActivationFunctionType
BF16 = ml_dtypes.bfloat16

S, B, D = 2048, 2, 1024
H, DK = 16, 64
NCORES = 8
HC = 4                 # heads per core
CH = HC * DK           # 256 local channels per core
SCALE = np.float32(1.0 / np.sqrt(DK))
GROUPS = [[0, 1, 2, 3], [4, 5, 6, 7]]

NKD = D // 128         # 8 contraction tiles for projections
NTK = S // 128         # 16 key tiles

# Tapered query blocks; each block is one ReduceScatter chunk.
TQS = [512, 512, 512, 512]
NB = len(TQS)
TQ0 = [sum(TQS[:i]) for i in range(NB)]
NSUB = [t // 128 for t in TQS]              # 128-row out subtiles per block
SUB0 = [t // 128 for t in TQ0]              # first subtile index of block


def build_nc():
    f32, bf16 = dt.float32, dt.bfloat16
    nc = bacc.Bacc("TRN2", target_bir_lowering=False, debug=False,
                   num_devices=NCORES)

    xq = nc.dram_tensor("xq_t", [D, S], bf16, kind="ExternalInput").ap()
    xk = nc.dram_tensor("xk_t", [D, S], bf16, kind="ExternalInput").ap()
    xv = nc.dram_tensor("xv_t", [D, S], bf16, kind="ExternalInput").ap()
    wq = nc.dram_tensor("wq_t", [D, CH], bf16, kind="ExternalInput").ap()
    wk = nc.dram_tensor("wk_t", [D, CH], bf16, kind="ExternalInput").ap()
    wv = nc.dram_tensor("wv_t", [D, CH], bf16, kind="ExternalInput").ap()
    wo = nc.dram_tensor("wo_t", [CH, D], bf16, kind="ExternalInput").ap()
    bq = nc.dram_tensor("bq", [2, 128], f32, kind="ExternalInput").ap()
    bk = nc.dram_tensor("bk", [2, 128], f32, kind="ExternalInput").ap()
    ones = nc.dram_tensor("ones", [128, HC], bf16, kind="ExternalInput").ap()
    # Chunk c covers global token rows [TQ0[c], TQ0[c]+TQS[c]);
    # group-rank j receives rows TQ0[c] + j*TQS[c]//4 onward.
    out_ext = nc.dram_tensor("out_rs", [S // 4, D], bf16,
                             kind="ExternalOutput").ap()

    with tile.TileContext(nc) as tc:
        with tc.tile_pool(name="const", bufs=1) as cp, \
             tc.tile_pool(name="stream", bufs=1) as sp, \
             tc.tile_pool(name="psum", bufs=1, space="PSUM") as pp, \
             tc.tile_pool(name="dram", bufs=1, space="DRAM") as dp:

            # ---- resident weights / biases (DMA in need-order: wq first) ----
            wq_sb = [cp.tile([128, CH], bf16, tag=f"wq{k}", name=f"wq{k}")
                     for k in range(NKD)]
            wk_sb = [cp.tile([128, CH], bf16, tag=f"wk{k}", name=f"wk{k}")
                     for k in range(NKD)]
            wv_sb = [cp.tile([128, CH], bf16, tag=f"wv{k}", name=f"wv{k}")
                     for k in range(NKD)]
            wo_sb = [cp.tile([128, D], bf16, tag=f"wo{k}", name=f"wo{k}")
                     for k in range(2)]
            bq_sb = [cp.tile([128, 1], f32, tag=f"bq{j}", name=f"bq{j}")
                     for j in range(2)]
            bk_sb = [cp.tile([128, 1], f32, tag=f"bk{j}", name=f"bk{j}")
                     for j in range(2)]
            for k in range(NKD):
                nc.scalar.dma_start(wq_sb[k][:], wq[k * 128:(k + 1) * 128, :])
            for j in range(2):
                nc.scalar.dma_start(bq_sb[j][:], bq[j].unsqueeze(1))
                nc.scalar.dma_start(bk_sb[j][:], bk[j].unsqueeze(1))
            for k in range(NKD):
                nc.scalar.dma_start(wk_sb[k][:], wk[k * 128:(k + 1) * 128, :])
            for k in range(NKD):
                nc.scalar.dma_start(wv_sb[k][:], wv[k * 128:(k + 1) * 128, :])
            for k in range(2):
                nc.scalar.dma_start(wo_sb[k][:], wo[k * 128:(k + 1) * 128, :])

            # ---- persistent activations ----
            qc = [cp.tile([128, S], bf16, tag=f"qc{j}", name=f"qc{j}")
                  for j in range(2)]
            kc = [cp.tile([128, S], bf16, tag=f"kc{j}", name=f"kc{j}")
                  for j in range(2)]
            # V tiles: [token128, 4*(64 V + 1 ones)] per key tile
            vt = [cp.tile([128, HC * (DK + 1)], bf16, tag=f"vt{t}",
                          name=f"vt{t}") for t in range(NTK)]
            ctx = [cp.tile([128, S], bf16, tag=f"ctx{j}", name=f"ctx{j}")
                   for j in range(2)]
            ones_sb = cp.tile([128, HC], bf16, tag="ones", name="ones_sb")
            nc.scalar.dma_start(ones_sb[:], ones[:])
            for t in range(NTK):
                vt_view = vt[t][:].rearrange("p (h c) -> p h c", h=HC)
                nc.vector.tensor_copy(vt_view[:, :, DK:DK + 1],
                                      ones_sb[:].unsqueeze(2))

            # ---- Q/K projections: psum[j-tile] [128ch, 512t] = sum_k
            #      wq[k][:, j]   (stationary) . xq[k, t512] (moving) ----
            TP = 512
            for x_dram, w_sb, b_sb, dst, dma_eng in (
                    (xq, wq_sb, bq_sb, qc, nc.sync),
                    (xk, wk_sb, bk_sb, kc, nc.scalar)):
                for th in range(2):            # halves of the token range
                    xts = []
                    for k in range(NKD):
                        for t in range(2):
                            xt = sp.tile([128, TP], bf16, tag="xs", bufs=20,
                                         name=f"xs{k}_{t}")
                            tq0 = (th * 2 + t) * TP
                            dma_eng.dma_start(
                                xt[:], x_dram[k * 128:(k + 1) * 128,
                                              tq0:tq0 + TP])
                            xts.append(xt)
                    for j in range(2):
                        ps = [pp.tile([128, TP], f32, tag="cx", bufs=2,
                                      name=f"pp{j}_{t}") for t in range(2)]
                        for k in range(NKD):
                            for t in range(2):
                                nc.tensor.matmul(
                                    ps[t][:],
                                    w_sb[k][:, j * 128:(j + 1) * 128],
                                    xts[2 * k + t][:],
                                    start=(k == 0), stop=(k == NKD - 1))
                        for t in range(2):
                            tq0 = (th * 2 + t) * TP
                            nc.scalar.activation(
                                dst[j][:, tq0:tq0 + TP], ps[t][:],
                                AF.Identity, bias=b_sb[j][:, 0:1])

            # ---- V projection: psum [128t, 256ch] = sum_k
            #      xv[k, t128] (stationary) . wv[k] (moving) ----
            for tt in range(4):                # big spans of 4 t-tiles
                xvts = []
                for k in range(NKD):
                    xvt_ = sp.tile([128, TP], bf16, tag="xs", bufs=20,
                                   name=f"xvt{k}")
                    nc.sync.dma_start(
                        xvt_[:], xv[k * 128:(k + 1) * 128,
                                    tt * TP:(tt + 1) * TP])
                    xvts.append(xvt_)
                for ts in range(4):
                    t = tt * 4 + ts
                    pv = pp.tile([128, CH], f32, tag="cx", bufs=2,
                                 name=f"pv{t}")
                    for k in range(NKD):
                        nc.tensor.matmul(
                            pv[:], xvts[k][:, ts * 128:(ts + 1) * 128],
                            wv_sb[k][:],
                            start=(k == 0), stop=(k == NKD - 1))
                    # strided copy into [128, 4, 65][:, :, 0:64]
                    dst_view = vt[t][:].rearrange("p (h c) -> p h c", h=HC)
                    src_view = pv[:].rearrange("p (h c) -> p h c", h=HC)
                    nc.vector.tensor_copy(dst_view[:, :, 0:DK], src_view)

            # ---- attention + output projection ----
            # Warmup collective: the first CC op on the stream pays a
            # ~15-25us warmup penalty; absorb it with a tiny dummy
            # ReduceScatter (contents irrelevant, output unused) issued at
            # kernel start so the real per-chunk ops run at steady state.
            cc_ins = [dp.tile([TQS[c], D], bf16, tag=f"ccin{c}",
                              name=f"cc_in{c}") for c in range(NB)]
            cc_warm_in = dp.tile([16, D], bf16, tag="ccwi", name="cc_warm_in")
            cc_warm_out = dp.tile([4, D], bf16, tag="ccwo",
                                  name="cc_warm_out")
            nc.gpsimd.collective_compute(
                "ReduceScatter", mybir.AluOpType.add,
                replica_groups=GROUPS,
                ins=[cc_warm_in[:]], outs=[cc_warm_out[:]])
            cc_outs = [dp.tile([TQS[c] // 4, D], bf16, tag=f"ccout{c}",
                               name=f"cc_out{c}") for c in range(NB)]

            def emit_outproj_subtile(sub, chunk):
                """Out-projection + store for one 128-row output subtile."""
                t0 = sub * 128
                po = pp.tile([128, 1024], f32, tag="s1", bufs=3,
                             name=f"po{sub}")
                for e in range(2):
                    for dv in range(2):
                        nc.tensor.matmul(
                            po[:, e * 512:(e + 1) * 512],
                            ctx[dv][:, t0:t0 + 128],
                            wo_sb[dv][:, e * 512:(e + 1) * 512],
                            start=(dv == 0), stop=(dv == 1))
                osb = sp.tile([128, D], bf16, tag="ot", bufs=8,
                              name=f"ot{sub}")
                nc.vector.tensor_copy(osb[:], po[:])
                r0 = sub * 128 - TQ0[chunk]
                nc.sync.dma_start(cc_ins[chunk][r0:r0 + 128, :], osb[:])
                if sub + 1 == SUB0[chunk] + NSUB[chunk]:
                    # chunk complete: ReduceScatter it (overlaps the
                    # attention compute of the following blocks)
                    nc.gpsimd.collective_compute(
                        "ReduceScatter", mybir.AluOpType.add,
                        replica_groups=GROUPS,
                        ins=[cc_ins[chunk][:]], outs=[cc_outs[chunk][:]])

            # Flattened attention stream over all (block, head-pair)
            # steps, with each PV pair deferred PV_LAG steps behind its
            # scores/exp. The deferral crosses pair boundaries, so the
            # next pair's scores+exp issue before the previous pair's
            # last PV and its cx-PSUM evacuation latency stays off the
            # ACT engine's critical path.
            PV_LAG = 2

            def emit_normalize(bi, p, cx):
                tq0, tqn = TQ0[bi], TQS[bi]
                cxs = []
                for h in range(2):
                    c_ = sp.tile([65, 512], f32, tag="cxs", bufs=4,
                                 name=f"cxs{p}_{h}")
                    nc.vector.tensor_copy(c_[:, 0:tqn], cx[h][:])
                    cxs.append(c_[:, 0:tqn])
                for h in range(2):
                    den = sp.tile([1, 512], f32, tag="den", bufs=4,
                                  name=f"den{p}_{h}")
                    nc.vector.tensor_copy(den[:, 0:tqn], cxs[h][64:65, :])
                    rc = sp.tile([1, 512], f32, tag="rc", bufs=4,
                                 name=f"rc{p}_{h}")
                    nc.vector.reciprocal_approx_fast(rc[:, 0:tqn],
                                                     den[:, 0:tqn])
                    bc = sp.tile([64, 512], f32, tag="bc", bufs=4,
                                 name=f"bc{p}_{h}")
                    nc.gpsimd.partition_broadcast(bc[:, 0:tqn],
                                                  rc[:, 0:tqn])
                    nc.vector.tensor_mul(
                        ctx[p][h * 64:(h + 1) * 64, tq0:tq0 + tqn],
                        cxs[h][0:64, :], bc[:, 0:tqn])

            pvq = []        # deferred PV steps: (bi, p, tk, etf, cx)

            def pop_pv():
                bi_, p_, tk_, etf_, cx_ = pvq.pop(0)
                tqn_ = TQS[bi_]
                for h in range(2):
                    hl = p_ * 2 + h
                    nc.tensor.matmul(
                        cx_[h][:],
                        vt[tk_][:, hl * 65:(hl + 1) * 65],
                        etf_[:, h * 512:h * 512 + tqn_],
                        start=(tk_ == 0), stop=(tk_ == NTK - 1))
                if tk_ == NTK - 1:
                    emit_normalize(bi_, p_, cx_)

            for bi in range(NB):
                tq0, tqn = TQ0[bi], TQS[bi]
                for p in range(2):             # head pairs (2p, 2p+1)
                    cxf = [pp.tile([65, 512], f32, tag="cx", bufs=2,
                                   name=f"cx{p}_{h}") for h in range(2)]
                    cx = [c_[:, 0:tqn] for c_ in cxf]
                    for tk in range(NTK):
                        # previous block's out-projection, interleaved a few
                        # steps into this block so its ctx (behind the
                        # normalize chain) is ready when the PE reaches it
                        if bi > 0 and p == 0 and tk >= 4 and \
                                (tk - 4) % 3 == 0:
                            j = (tk - 4) // 3
                            if j < NSUB[bi - 1]:
                                emit_outproj_subtile(SUB0[bi - 1] + j, bi - 1)
                        # head h at col offset 512*h: every matmul PSUM
                        # output starts on a 2KB bank boundary
                        s1f = pp.tile([128, 1024], f32, tag="s1", bufs=3,
                                      name=f"s1{tk}")
                        etf = sp.tile([128, 1024], bf16, tag="et", bufs=6,
                                      name=f"et{tk}")
                        for h in range(2):      # adjacent -> row-pack overlap
                            r0 = h * 64
                            nc.tensor.matmul(
                                s1f[:, h * 512:h * 512 + tqn],
                                kc[p][r0:r0 + 64, tk * 128:(tk + 1) * 128],
                                qc[p][r0:r0 + 64, tq0:tq0 + tqn],
                                start=True, stop=True)
                        if tqn == 512:
                            nc.scalar.activation(etf[:], s1f[:], AF.Exp)
                        else:
                            for h in range(2):
                                nc.scalar.activation(
                                    etf[:, h * 512:h * 512 + tqn],
                                    s1f[:, h * 512:h * 512 + tqn], AF.Exp)
                        pvq.append((bi, p, tk, etf, cx))
                        while len(pvq) > PV_LAG:
                            pop_pv()
            while pvq:
                pop_pv()
            # last block's out-projection
            for j in range(NSUB[NB - 1]):
                emit_outproj_subtile(SUB0[NB - 1] + j, NB - 1)

            # final stores, force-scheduled at the very end so a store
            # waiting on its ReduceScatter never head-of-line-blocks the
            # sync DMA queue mid-kernel
            with tc.tile_wait_until(10):
                for c in range(NB):
                    o0 = TQ0[c] // 4
                    nc.sync.dma_start(out_ext[o0:o0 + TQS[c] // 4, :],
                                      cc_outs[c][:])

    nc.finalize()
    return nc


_NC = None


def _get_nc():
    global _NC
    if _NC is None:
        _NC = build_nc()
    return _NC


def make_in_maps(q, k, v, Wq, bq, Wk, bk, Wv, bv, Wo, bo):
    """Shard + precondition full inputs into per-core input maps."""
    xq_b = [np.ascontiguousarray(q[:, b, :].T).astype(BF16) for b in range(B)]
    xk_b = [np.ascontiguousarray(k[:, b, :].T).astype(BF16) for b in range(B)]
    xv_b = [np.ascontiguousarray(v[:, b, :].T).astype(BF16) for b in range(B)]
    in_maps = []
    for r in range(NCORES):
        b = r // 4
        g = r % 4
        ch = slice(g * CH, (g + 1) * CH)
        in_maps.append({
            "xq_t": xq_b[b], "xk_t": xk_b[b], "xv_t": xv_b[b],
            "wq_t": np.ascontiguousarray((Wq[ch, :] * SCALE).T).astype(BF16),
            "wk_t": np.ascontiguousarray(Wk[ch, :].T).astype(BF16),
            "wv_t": np.ascontiguousarray(Wv[ch, :].T).astype(BF16),
            "wo_t": np.ascontiguousarray(Wo[:, ch].T).astype(BF16),
            "bq": (bq[ch] * SCALE).reshape(2, 128).astype(np.float32),
            "bk": bk[ch].reshape(2, 128).astype(np.float32),
            "ones": np.ones((128, HC), dtype=BF16),
        })
    return in_maps


def assemble(results, Wo, bv, bo):
    """Gather per-core ReduceScatter slices into the full [S, B, D] output."""
    out = np.empty((S, B, D), dtype=np.float32)
    for r in range(NCORES):
        b = r // 4
        j = r % 4
        for c in range(NB):
            rows = TQS[c] // 4
            g0 = TQ0[c] + j * rows               # global token rows
            o0 = TQ0[c] // 4                     # rows within out_rs
            out[g0:g0 + rows, b, :] = \
                results[r]["out_rs"][o0:o0 + rows].astype(np.float32)
    out += (bo + Wo @ bv).astype(np.float32)
    return out


def run_sharded(inputs, trace=False):
    nc = _get_nc()
    in_maps = make_in_maps(**inputs)
    res = run_bass_kernel_spmd(nc, in_maps, list(range(NCORES)), trace=trace)
    full = assemble(res.results, np.asarray(inputs["Wo"], dtype=np.float32),
                    np.asarray(inputs["bv"], dtype=np.float32),
                    np.asarray(inputs["bo"], dtype=np.float32))
    return full, res


def kernel(**inputs) -> np.ndarray:
    inputs = {k_: np.asarray(v_, dtype=np.float32)
              for k_, v_ in inputs.items()}
    full, _ = run_sharded(inputs)
    return full


# revision 21
# speedup vs baseline: 1.0662x; 1.0387x over previous
"""Multi-head attention (S=2048, B=2, D=1024, H=16) on 8 Trainium2 cores.

Sharding: tensor-parallel over heads (4 groups of 4 heads) x data-parallel
over batch (2). Core r handles batch r//4, heads [4*(r%4), 4*(r%4)+4).
Each core projects its 256 channels, runs attention for its 4 heads, applies
its slice of the output projection, and a ReduceScatter over each 4-core
batch group sums the partial outputs and leaves each core with a 512-row
slice of the final [2048, 1024] output.

All matmul operands are bf16 (fp32r measures ~2 cycles/column on this HW;
bf16 measures ~1), with fp32 PSUM accumulation. Softmax denominators come
free from an extra ones-column appended to V in the PV matmul. V's bias and
the output bias are folded out algebraically and added on the host.

The query blocks taper (512,512,512,256,128,128) so the trailing
ReduceScatter chunks shrink: the only collective that cannot overlap
compute is the final 128-row one.
"""
import sys

sys.path.insert(0, "/opt/trn_rl_repo")

import numpy as np
import ml_dtypes
import concourse.bacc as bacc
import concourse.mybir as mybir
from concourse import tile
from concourse.bass_utils import run_bass_kernel_spmd

dt = mybir.dt
AF = mybir.ActivationFunctionType
BF16 = ml_dtypes.bfloat16

S, B, D = 2048, 2, 1024
H, DK = 16, 64
NCORES = 8
HC = 4                 # heads per core
CH = HC * DK           # 256 local channels per core
SCALE = np.float32(1.0 / np.sqrt(DK))
GROUPS = [[0, 1, 2, 3], [4, 5, 6, 7]]

NKD = D // 128         # 8 contraction tiles for projections
NTK = S // 128         # 16 key tiles

# Tapered query blocks; each block is one ReduceScatter chunk.
TQS = [512, 512, 512, 512]
NB = len(TQS)
TQ0 = [sum(TQS[:i]) for i in range(NB)]
NSUB = [t // 128 for t in TQS]              # 128-row out subtiles per block
SUB0 = [t // 128 for t in TQ0]              # first subtile index of block


def build_nc():
    f32, bf16 = dt.float32, dt.bfloat16
    nc = bacc.Bacc("TRN2", target_bir_lowering=False, debug=False,
                   num_devices=NCORES)

    xq = nc.dram_tensor("xq_t", [D, S], bf16, kind="ExternalInput").ap()
    xk = nc.dram_tensor("xk_t", [D, S], bf16, kind="ExternalInput").ap()
    xv = nc.dram_tensor("xv_t", [D, S], bf16, kind="ExternalInput").ap()
    wq = nc.dram_tensor("wq_t", [D, CH], bf16, kind="ExternalInput").ap()
    wk = nc.dram_tensor("wk_t", [D, CH], bf16, kind="ExternalInput").ap()
    wv = nc.dram_tensor("wv_t", [D, CH], bf16, kind="ExternalInput").ap()
    wo = nc.dram_tensor("wo_t", [CH, D], bf16, kind="ExternalInput").ap()
    bq = nc.dram_tensor("bq", [2, 128], f32, kind="ExternalInput").ap()
    bk = nc.dram_tensor("bk", [2, 128], f32, kind="ExternalInput").ap()
    ones = nc.dram_tensor("ones", [128, HC], bf16, kind="ExternalInput").ap()
    # Chunk c covers global token rows [TQ0[c], TQ0[c]+TQS[c]);
    # group-rank j receives rows TQ0[c] + j*TQS[c]//4 onward.
    out_ext = nc.dram_tensor("out_rs", [S // 4, D], bf16,
                             kind="ExternalOutput").ap()

    with tile.TileContext(nc) as tc:
        with tc.tile_pool(name="const", bufs=1) as cp, \
             tc.tile_pool(name="stream", bufs=1) as sp, \
             tc.tile_pool(name="psum", bufs=1, space="PSUM") as pp, \
             tc.tile_pool(name="dram", bufs=1, space="DRAM") as dp:

            # ---- resident weights / biases (DMA in need-order: wq first) ----
            wq_sb = [cp.tile([128, CH], bf16, tag=f"wq{k}", name=f"wq{k}")
                     for k in range(NKD)]
            wk_sb = [cp.tile([128, CH], bf16, tag=f"wk{k}", name=f"wk{k}")
                     for k in range(NKD)]
            wv_sb = [cp.tile([128, CH], bf16, tag=f"wv{k}", name=f"wv{k}")
                     for k in range(NKD)]
            wo_sb = [cp.tile([128, D], bf16, tag=f"wo{k}", name=f"wo{k}")
                     for k in range(2)]
            bq_sb = [cp.tile([128, 1], f32, tag=f"bq{j}", name=f"bq{j}")
                     for j in range(2)]
            bk_sb = [cp.tile([128, 1], f32, tag=f"bk{j}", name=f"bk{j}")
                     for j in range(2)]
            for k in range(NKD):
                nc.scalar.dma_start(wq_sb[k][:], wq[k * 128:(k + 1) * 128, :])
            for j in range(2):
                nc.scalar.dma_start(bq_sb[j][:], bq[j].unsqueeze(1))
                nc.scalar.dma_start(bk_sb[j][:], bk[j].unsqueeze(1))
            for k in range(NKD):
                nc.scalar.dma_start(wk_sb[k][:], wk[k * 128:(k + 1) * 128, :])
            for k in range(NKD):
                nc.scalar.dma_start(wv_sb[k][:], wv[k * 128:(k + 1) * 128, :])
            for k in range(2):
                nc.scalar.dma_start(wo_sb[k][:], wo[k * 128:(k + 1) * 128, :])

            # ---- persistent activations ----
            qc = [cp.tile([128, S], bf16, tag=f"qc{j}", name=f"qc{j}")
                  for j in range(2)]
            kc = [cp.tile([128, S], bf16, tag=f"kc{j}", name=f"kc{j}")
                  for j in range(2)]
            # V tiles: [token128, 4*(64 V + 1 ones)] per key tile
            vt = [cp.tile([128, HC * (DK + 1)], bf16, tag=f"vt{t}",
                          name=f"vt{t}") for t in range(NTK)]
            ctx = [cp.tile([128, S], bf16, tag=f"ctx{j}", name=f"ctx{j}")
                   for j in range(2)]
            ones_sb = cp.tile([128, HC], bf16, tag="ones", name="ones_sb")
            nc.scalar.dma_start(ones_sb[:], ones[:])
            for t in range(NTK):
                vt_view = vt[t][:].rearrange("p (h c) -> p h c", h=HC)
                nc.vector.tensor_copy(vt_view[:, :, DK:DK + 1],
                                      ones_sb[:].unsqueeze(2))

            # ---- projections, ordered so attention's inputs land first:
            # Q tokens 0:1024 (block 0/1), K (scalar-queue-fed), V, then
            # Q tokens 1024:2048 whose DMA hides under V-proj compute ----
            TP = 512

            def qk_proj_half(x_dram, w_sb, b_sb, dst, dma_eng, th):
                xts = []
                for k in range(NKD):
                    for t in range(2):
                        xt = sp.tile([128, TP], bf16, tag="xs", bufs=20,
                                     name=f"xs{k}_{t}")
                        tq0 = (th * 2 + t) * TP
                        dma_eng.dma_start(
                            xt[:], x_dram[k * 128:(k + 1) * 128,
                                          tq0:tq0 + TP])
                        xts.append(xt)
                for j in range(2):
                    ps = [pp.tile([128, TP], f32, tag="cx", bufs=2,
                                  name=f"pp{j}_{t}") for t in range(2)]
                    for k in range(NKD):
                        for t in range(2):
                            nc.tensor.matmul(
                                ps[t][:],
                                w_sb[k][:, j * 128:(j + 1) * 128],
                                xts[2 * k + t][:],
                                start=(k == 0), stop=(k == NKD - 1))
                    for t in range(2):
                        tq0 = (th * 2 + t) * TP
                        nc.scalar.activation(
                            dst[j][:, tq0:tq0 + TP], ps[t][:],
                            AF.Identity, bias=b_sb[j][:, 0:1])

            qk_proj_half(xq, wq_sb, bq_sb, qc, nc.sync, 0)
            for th in range(2):
                qk_proj_half(xk, wk_sb, bk_sb, kc, nc.scalar, th)

            # ---- V projection: psum [128t, 256ch] = sum_k
            #      xv[k, t128] (stationary) . wv[k] (moving) ----
            for tt in range(4):                # big spans of 4 t-tiles
                xvts = []
                for k in range(NKD):
                    xvt_ = sp.tile([128, TP], bf16, tag="xs", bufs=20,
                                   name=f"xvt{k}")
                    nc.sync.dma_start(
                        xvt_[:], xv[k * 128:(k + 1) * 128,
                                    tt * TP:(tt + 1) * TP])
                    xvts.append(xvt_)
                for ts in range(4):
                    t = tt * 4 + ts
                    pv = pp.tile([128, CH], f32, tag="cx", bufs=2,
                                 name=f"pv{t}")
                    for k in range(NKD):
                        nc.tensor.matmul(
                            pv[:], xvts[k][:, ts * 128:(ts + 1) * 128],
                            wv_sb[k][:],
                            start=(k == 0), stop=(k == NKD - 1))
                    # strided copy into [128, 4, 65][:, :, 0:64]
                    dst_view = vt[t][:].rearrange("p (h c) -> p h c", h=HC)
                    src_view = pv[:].rearrange("p (h c) -> p h c", h=HC)
                    nc.vector.tensor_copy(dst_view[:, :, 0:DK], src_view)

            qk_proj_half(xq, wq_sb, bq_sb, qc, nc.sync, 1)

            # ---- attention + output projection ----
            # Warmup collective: the first CC op on the stream pays a
            # ~15-25us warmup penalty; absorb it with a tiny dummy
            # ReduceScatter (contents irrelevant, output unused) issued at
            # kernel start so the real per-chunk ops run at steady state.
            cc_ins = [dp.tile([TQS[c], D], bf16, tag=f"ccin{c}",
                              name=f"cc_in{c}") for c in range(NB)]
            cc_warm_in = dp.tile([16, D], bf16, tag="ccwi", name="cc_warm_in")
            cc_warm_out = dp.tile([4, D], bf16, tag="ccwo",
                                  name="cc_warm_out")
            nc.gpsimd.collective_compute(
                "ReduceScatter", mybir.AluOpType.add,
                replica_groups=GROUPS,
                ins=[cc_warm_in[:]], outs=[cc_warm_out[:]])
            cc_outs = [dp.tile([TQS[c] // 4, D], bf16, tag=f"ccout{c}",
                               name=f"cc_out{c}") for c in range(NB)]

            def emit_outproj_subtile(sub, chunk):
                """Out-projection + store for one 128-row output subtile."""
                t0 = sub * 128
                po = pp.tile([128, 1024], f32, tag="s1", bufs=3,
                             name=f"po{sub}")
                for e in range(2):
                    for dv in range(2):
                        nc.tensor.matmul(
                            po[:, e * 512:(e + 1) * 512],
                            ctx[dv][:, t0:t0 + 128],
                            wo_sb[dv][:, e * 512:(e + 1) * 512],
                            start=(dv == 0), stop=(dv == 1))
                osb = sp.tile([128, D], bf16, tag="ot", bufs=8,
                              name=f"ot{sub}")
                nc.vector.tensor_copy(osb[:], po[:])
                r0 = sub * 128 - TQ0[chunk]
                nc.sync.dma_start(cc_ins[chunk][r0:r0 + 128, :], osb[:])
                if sub + 1 == SUB0[chunk] + NSUB[chunk]:
                    # chunk complete: ReduceScatter it (overlaps the
                    # attention compute of the following blocks)
                    nc.gpsimd.collective_compute(
                        "ReduceScatter", mybir.AluOpType.add,
                        replica_groups=GROUPS,
                        ins=[cc_ins[chunk][:]], outs=[cc_outs[chunk][:]])

            # Flattened attention stream over all (block, head-pair)
            # steps, with each PV pair deferred PV_LAG steps behind its
            # scores/exp. The deferral crosses pair boundaries, so the
            # next pair's scores+exp issue before the previous pair's
            # last PV and its cx-PSUM evacuation latency stays off the
            # ACT engine's critical path.
            PV_LAG = 2

            def emit_normalize(bi, p, cx):
                tq0, tqn = TQ0[bi], TQS[bi]
                cxs = []
                for h in range(2):
                    c_ = sp.tile([65, 512], f32, tag="cxs", bufs=4,
                                 name=f"cxs{p}_{h}")
                    nc.vector.tensor_copy(c_[:, 0:tqn], cx[h][:])
                    cxs.append(c_[:, 0:tqn])
                for h in range(2):
                    den = sp.tile([1, 512], f32, tag="den", bufs=4,
                                  name=f"den{p}_{h}")
                    nc.vector.tensor_copy(den[:, 0:tqn], cxs[h][64:65, :])
                    rc = sp.tile([1, 512], f32, tag="rc", bufs=4,
                                 name=f"rc{p}_{h}")
                    nc.vector.reciprocal_approx_fast(rc[:, 0:tqn],
                                                     den[:, 0:tqn])
                    bc = sp.tile([64, 512], f32, tag="bc", bufs=4,
                                 name=f"bc{p}_{h}")
                    nc.gpsimd.partition_broadcast(bc[:, 0:tqn],
                                                  rc[:, 0:tqn])
                    nc.vector.tensor_mul(
                        ctx[p][h * 64:(h + 1) * 64, tq0:tq0 + tqn],
                        cxs[h][0:64, :], bc[:, 0:tqn])

            pvq = []        # deferred PV steps: (bi, p, tk, etf, cx)

            def pop_pv():
                bi_, p_, tk_, etf_, cx_ = pvq.pop(0)
                tqn_ = TQS[bi_]
                for h in range(2):
                    hl = p_ * 2 + h
                    nc.tensor.matmul(
                        cx_[h][:],
                        vt[tk_][:, hl * 65:(hl + 1) * 65],
                        etf_[:, h * 512:h * 512 + tqn_],
                        start=(tk_ == 0), stop=(tk_ == NTK - 1))
                if tk_ == NTK - 1:
                    emit_normalize(bi_, p_, cx_)

            for bi in range(NB):
                tq0, tqn = TQ0[bi], TQS[bi]
                for p in range(2):             # head pairs (2p, 2p+1)
                    cxf = [pp.tile([65, 512], f32, tag="cx", bufs=2,
                                   name=f"cx{p}_{h}") for h in range(2)]
                    cx = [c_[:, 0:tqn] for c_ in cxf]
                    for tk in range(NTK):
                        # previous block's out-projection, interleaved a few
                        # steps into this block so its ctx (behind the
                        # normalize chain) is ready when the PE reaches it
                        if bi > 0 and p == 0 and tk >= 4 and \
                                (tk - 4) % 3 == 0:
                            j = (tk - 4) // 3
                            if j < NSUB[bi - 1]:
                                emit_outproj_subtile(SUB0[bi - 1] + j, bi - 1)
                        # head h at col offset 512*h: every matmul PSUM
                        # output starts on a 2KB bank boundary
                        s1f = pp.tile([128, 1024], f32, tag="s1", bufs=3,
                                      name=f"s1{tk}")
                        etf = sp.tile([128, 1024], bf16, tag="et", bufs=6,
                                      name=f"et{tk}")
                        for h in range(2):      # adjacent -> row-pack overlap
                            r0 = h * 64
                            nc.tensor.matmul(
                                s1f[:, h * 512:h * 512 + tqn],
                                kc[p][r0:r0 + 64, tk * 128:(tk + 1) * 128],
                                qc[p][r0:r0 + 64, tq0:tq0 + tqn],
                                start=True, stop=True)
                        if tqn == 512:
                            nc.scalar.activation(etf[:], s1f[:], AF.Exp)
                        else:
                            for h in range(2):
                                nc.scalar.activation(
                                    etf[:, h * 512:h * 512 + tqn],
                                    s1f[:, h * 512:h * 512 + tqn], AF.Exp)
                        pvq.append((bi, p, tk, etf, cx))
                        while len(pvq) > PV_LAG:
                            pop_pv()
            while pvq:
                pop_pv()
            # last block's out-projection
            for j in range(NSUB[NB - 1]):
                emit_outproj_subtile(SUB0[NB - 1] + j, NB - 1)

            # final stores, force-scheduled at the very end so a store
            # waiting on its ReduceScatter never head-of-line-blocks the
            # sync DMA queue mid-kernel
            with tc.tile_wait_until(10):
                for c in range(NB):
                    o0 = TQ0[c] // 4
                    nc.sync.dma_start(out_ext[o0:o0 + TQS[c] // 4, :],
                                      cc_outs[c][:])

    nc.finalize()
    return nc


_NC = None


def _get_nc():
    global _NC
    if _NC is None:
        _NC = build_nc()
    return _NC


def make_in_maps(q, k, v, Wq, bq, Wk, bk, Wv, bv, Wo, bo):
    """Shard + precondition full inputs into per-core input maps."""
    xq_b = [np.ascontiguousarray(q[:, b, :].T).astype(BF16) for b in range(B)]
    xk_b = [np.ascontiguousarray(k[:, b, :].T).astype(BF16) for b in range(B)]
    xv_b = [np.ascontiguousarray(v[:, b, :].T).astype(BF16) for b in range(B)]
    in_maps = []
    for r in range(NCORES):
        b = r // 4
        g = r % 4
        ch = slice(g * CH, (g + 1) * CH)
        in_maps.append({
            "xq_t": xq_b[b], "xk_t": xk_b[b], "xv_t": xv_b[b],
            "wq_t": np.ascontiguousarray((Wq[ch, :] * SCALE).T).astype(BF16),
            "wk_t": np.ascontiguousarray(Wk[ch, :].T).astype(BF16),
            "wv_t": np.ascontiguousarray(Wv[ch, :].T).astype(BF16),
            "wo_t": np.ascontiguousarray(Wo[:, ch].T).astype(BF16),
            "bq": (bq[ch] * SCALE).reshape(2, 128).astype(np.float32),
            "bk": bk[ch].reshape(2, 128).astype(np.float32),
            "ones": np.ones((128, HC), dtype=BF16),
        })
    return in_maps


def assemble(results, Wo, bv, bo):
    """Gather per-core ReduceScatter slices into the full [S, B, D] output."""
    out = np.empty((S, B, D), dtype=np.float32)
    for r in range(NCORES):
        b = r // 4
        j = r % 4
        for c in range(NB):
            rows = TQS[c] // 4
            g0 = TQ0[c] + j * rows               # global token rows
            o0 = TQ0[c] // 4                     # rows within out_rs
            out[g0:g0 + rows, b, :] = \
                results[r]["out_rs"][o0:o0 + rows].astype(np.float32)
    out += (bo + Wo @ bv).astype(np.float32)
    return out


def run_sharded(inputs, trace=False):
    nc = _get_nc()
    in_maps = make_in_maps(**inputs)
    res = run_bass_kernel_spmd(nc, in_maps, list(range(NCORES)), trace=trace)
    full = assemble(res.results, np.asarray(inputs["Wo"], dtype=np.float32),
                    np.asarray(inputs["bv"], dtype=np.float32),
                    np.asarray(inputs["bo"], dtype=np.float32))
    return full, res


def kernel(**inputs) -> np.ndarray:
    inputs = {k_: np.asarray(v_, dtype=np.float32)
              for k_, v_ in inputs.items()}
    full, _ = run_sharded(inputs)
    return full
